# revision 22
# baseline (speedup 1.0000x reference)
"""Trainium2 Bass kernel: float32 -> 32-channel bit-plane encoding.

For input x [4096, 512] f32, produces out [4096, 512, 32] f32 where
out[b, f, 0] = (x[b,f] < 0) and out[b, f, 1+j] = bit (30-j) of
bitcast_int32(|x[b,f]|), MSB first.

Wire-format design: every output element is exactly 0.0 or 1.0, so the
device computes and stores each of the 67M output elements as a uint8
{0,1}; the host applies a value-preserving widening cast to f32.  This
cuts device HBM write traffic 4x (8MB/core instead of 32MB/core), which
is the binding roofline (per-NeuronCore HBM/fabric bandwidth ~430 GB/s
observed).

Host-side repack makes the device compute uniform:
  i' = (bitcast_u32(x) & 0x7FFFFFFF) | ((x < 0) << 31)
stored as a big-endian byte stream, viewed as uint16 pairs.  Then output
channel k of feature f equals bit (7 - k%8) of stream byte 4f + k//8.

Device compute (VectorE), one fused tensor_scalar op per bit plane:
  plane_m = (x_u16 >> (7-m)) & 0x0101     m = 0..7
Each uint16 element yields TWO planar output bytes; the dense step-1
16-bit single-src pattern hits the DVE 4x perf mode; op duration
follows (58 + FD/4)/0.96GHz + ~90ns dispatch, ~84ns overlapped.

Measured critical path = preamble (7.3us, fixed) + first input receipt
(~3us after issue; first DMA per HWDGE ring only -- the second DMA on
a ring sees ~4.6us receipt) + vector stream + last output-piece issue +
teardown (~1.5us).  Schedule: the 4 row tiles form two column-merged
pair sections (rt01 / rt23, FD=2048 ops, ~8.3us total busy); each
section's two input DMAs are the FIRST DMA on their ring, so section 0
starts on two parallel single-receipt latencies and section 1's inputs
arrive during section 0.  Output pieces are few and large (issue cost
is ~128 descriptors per piece regardless of size): 5 pieces alternate
across the rings, the last gated by a small FD=512 op.

The planes land in HBM planar per SBUF partition; the host interleaves
planes/sections into [rows, F, 32] during the f32 cast.

Sharded row-wise over 8 NeuronCores (512 rows each, 4 row tiles of 128).
"""

import sys

if "/opt/trn_rl_repo" not in sys.path:
    sys.path.insert(0, "/opt/trn_rl_repo")

import numpy as np

import concourse.bass as bass
import concourse.mybir as mybir

P = 128           # SBUF partitions
F = 512           # features per row
K = 32            # output channels per feature
N_CORES = 8
ROWS_TOTAL = 4096
ROWS = ROWS_TOTAL // N_CORES   # rows per core (512)
NRT = ROWS // P                # row tiles per core (4)
W16 = F * 2                    # uint16 words per row (1024)
PW = 2 * W16                   # pair-section width (2048 u16)
PLANES = 8                     # bit planes per byte
OWS = PLANES * PW              # output u16 per partition per section
OCOLS = 2 * OWS                # output dram columns per partition (32768)
SPLIT = PW - 512               # last-plane split point (u16 cols)


def build_nc() -> bass.Bass:
    nc = bass.Bass("TRN2", target_bir_lowering=False, debug=False)
    u16 = mybir.dt.uint16

    xin = nc.declare_dram_parameter("xin", [ROWS, W16], u16, isOutput=False)
    out = nc.declare_dram_parameter("out", [P, OCOLS], u16, isOutput=True)
    xin_ap, out_ap = xin.ap(), out.ap()

    shift_and = (mybir.AluOpType.logical_shift_right,
                 mybir.AluOpType.bitwise_and)

    # vector ops: (section, m, lo, hi); section 0 = rt01, 1 = rt23.
    # Section 0 planes 0-1 are split at the rt0/rt1 boundary: the two
    # rt0 halves (gated on in0 alone) give ~850ns of work to absorb the
    # receipt divergence between the two input rings, so compute starts
    # on whichever ring lands FIRST and rarely bubbles on the other.
    vops = [(0, 0, 0, W16),                                  # ts 1 (rt0)
            (0, 1, 0, W16),                                  # ts 2 (rt0)
            (0, 0, W16, PW),                                 # ts 3 (rt1)
            (0, 1, W16, PW)]                                 # ts 4 (rt1)
    vops += [(0, m, 0, PW) for m in range(2, PLANES)]        # ts 5..10
    vops += [(1, m, 0, PW) for m in range(PLANES - 1)]       # ts 11..17
    vops += [(1, PLANES - 1, 0, SPLIT),                      # ts 18
             (1, PLANES - 1, SPLIT, PW)]                     # ts 19
    # out pieces: (engine 0=sync/1=scalar, sec, u16 lo, hi, ts_count).
    # Issue cost is ~128 descriptors per piece regardless of size, so
    # few large pieces; the final piece is small and last-gated.
    b7 = (PLANES - 1) * PW
    pieces = [
        (0, 0, 0, OWS, 10),                      # section 0 (2MB)
        (1, 1, 0, 3 * PW, 13),                   # sec1 planes 0-2 (1.5MB)
        (0, 1, 3 * PW, 6 * PW, 16),              # sec1 planes 3-5 (1.5MB)
        (1, 1, 6 * PW, b7 + SPLIT, 18),          # plane 6 + 7a (0.875MB)
        (0, 1, b7 + SPLIT, b7 + PW, 19),         # plane 7b (0.125MB)
    ]

    from contextlib import ExitStack
    with ExitStack() as ctx:
        xt = [ctx.enter_context(nc.sbuf_tensor(f"xt{s}", [P, PW], u16))
              for s in range(2)]
        ot = [ctx.enter_context(nc.sbuf_tensor(f"ot{s}", [P, OWS], u16))
              for s in range(2)]

        in_sem = [ctx.enter_context(nc.semaphore(f"in_sem{b}"))
                  for b in range(NRT)]
        ts_sem = ctx.enter_context(nc.semaphore("ts_sem"))
        od_sem = ctx.enter_context(nc.semaphore("od_sem"))

        ctx.enter_context(nc.Block(no_gpsimd_drain=True))
        block = nc.cur_block

        @block.vector
        def _(vec: bass.BassEngine):
            for i, (sec, m, lo, hi) in enumerate(vops):
                if i == 0:
                    vec.wait_ge(in_sem[0], 16)
                elif i == 2:
                    vec.wait_ge(in_sem[1], 16)
                elif i == PLANES + 2:
                    vec.wait_ge(in_sem[2], 16)
                    vec.wait_ge(in_sem[3], 16)
                vec.tensor_scalar(
                    ot[sec][:, m * PW + lo:m * PW + hi],
                    xt[sec][:, lo:hi],
                    7 - m,
                    0x0101,
                    *shift_and,
                ).then_inc(ts_sem)

        def piece_dma(eng, sec, lo, hi, n):
            eng.wait_ge(ts_sem, n)
            eng.dma_start(
                out_ap[:, sec * OWS + lo:sec * OWS + hi],
                ot[sec][:, lo:hi],
            ).then_inc(od_sem, 16)

        @block.sync
        def _(sp: bass.BassEngine):
            # rt0 -> xt0 lower half (first on ring), rt2 -> xt1 lower
            sp.dma_start(xt[0][:, 0:W16],
                         xin_ap[0:P, :]).then_inc(in_sem[0], 16)
            sp.dma_start(xt[1][:, 0:W16],
                         xin_ap[2 * P:3 * P, :]).then_inc(in_sem[2], 16)
            for (eng, sec, lo, hi, n) in pieces:
                if eng == 0:
                    piece_dma(sp, sec, lo, hi, n)

        @block.scalar
        def _(sc: bass.BassEngine):
            # rt1 -> xt0 upper half (first on ring), rt3 -> xt1 upper
            sc.dma_start(xt[0][:, W16:PW],
                         xin_ap[P:2 * P, :]).then_inc(in_sem[1], 16)
            sc.dma_start(xt[1][:, W16:PW],
                         xin_ap[3 * P:4 * P, :]).then_inc(in_sem[3], 16)
            for (eng, sec, lo, hi, n) in pieces:
                if eng == 1:
                    piece_dma(sc, sec, lo, hi, n)

    return nc


_NC_CACHE = None


def _get_nc():
    global _NC_CACHE
    if _NC_CACHE is None:
        _NC_CACHE = build_nc()
    return _NC_CACHE


def pack_shard(x_shard: np.ndarray) -> np.ndarray:
    """[ROWS, F] f32 -> [ROWS, W16] uint16: sign-normalized bitcast words
    as a big-endian byte stream, viewed as little-endian uint16 pairs."""
    x_shard = np.ascontiguousarray(x_shard)
    xi = (x_shard.view(np.uint32) & np.uint32(0x7FFFFFFF)) | \
        ((x_shard < 0).astype(np.uint32) << np.uint32(31))
    return xi.byteswap().view(np.uint16)


def unpack_shard(raw: np.ndarray) -> np.ndarray:
    """[P, OCOLS] uint16 planar pair-sections -> [ROWS, F, K] f32.

    Section s covers row tiles (2s, 2s+1): bytes [p, sec, m,
    rt_in_pair, 4f+j] -> out[(2*sec+rt)*128+p, f, 8j+m].
    """
    b = raw.view(np.uint8).reshape(P, 2, PLANES, 2, F, 4)
    r = b.transpose(1, 3, 0, 4, 5, 2).reshape(ROWS, F, K)
    return r.astype(np.float32)


def kernel(x: np.ndarray) -> np.ndarray:
    from concourse.bass_utils import run_bass_kernel_spmd

    x = np.asarray(x, dtype=np.float32)
    assert x.shape == (ROWS_TOTAL, F), x.shape
    nc = _get_nc()
    in_maps = [
        {"xin": pack_shard(x[i * ROWS:(i + 1) * ROWS])} for i in range(N_CORES)
    ]
    res = run_bass_kernel_spmd(nc, in_maps, list(range(N_CORES)))
    parts = [unpack_shard(res.results[i]["out"]) for i in range(N_CORES)]
    return np.concatenate(parts, axis=0)
